# revision 8
# baseline (speedup 1.0000x reference)
"""GNN message-passing (segment-mean + 3-layer MLP) Trainium2 kernel.

Strategy (8 NeuronCores, SPMD, full inputs in / full output out):
  - Host: assign nodes to 400 blocks of 128 slots (degree-balanced snake) so
    every block's incoming-edge count fits a fixed 12 k-tiles of 128 edges.
    Blocks 0-49 -> core 0, etc.  Edges are bucketed per receiver block,
    pre-scaled by 1/deg(recv), cast to bf16, and laid out [eslot, ktile*feat]
    so per-block DMAs are contiguous.  The one-hot scatter masks (0/1, fp8)
    are precomputed on the host as well.
  - Device per core: segment-mean as mask matmuls on the TensorEngine
    (12 k-tiles accumulated per 128-node block), then the 3-layer MLP over
    512-node chunks in feature-major layout with float32r matmuls
    (full-rate fp32-storage), relu+bias fused on the ScalarEngine, psum
    copies / final bias on the VectorEngine.  Edge/mask slabs stream on the
    SP HWDGE ring while weights/x stream on the ACT ring so the first
    scatter matmul is not queued behind the persistent loads.
    Output stays transposed; host untransposes + un-permutes.
"""
import sys

sys.path.insert(0, "/opt/trn_rl_repo")

import numpy as np
import ml_dtypes

from concourse import bacc
import concourse.mybir as mybir
import concourse.tile as tile
from concourse.bass_utils import run_bass_kernel_spmd

# problem shape (hardcoded per contract)
N_NODES = 50000
N_EDGES = 600000
D = 128          # node/edge feature dim
DH = 512         # hidden dim
C = 8            # cores
BPC = 50         # node blocks per core
NB = C * BPC     # 400 blocks total
SLOTS = BPC * 128  # 6400 node slots per core
T_BLK = 12       # edge k-tiles (128 edges) per block
TT = BPC * T_BLK   # k-tiles per core
CHUNKS = [2, 2, 2, 2] + [4] * 10 + [2]  # blocks per MLP chunk

F32 = mybir.dt.float32
F32R = mybir.dt.float32r
BF16 = mybir.dt.bfloat16
FP8 = mybir.dt.float8e4

_prog_cache = {}
LAST_RESULTS = None  # BassKernelResults of the most recent run (for test.py)


def _build_program():
    if "nc" in _prog_cache:
        return _prog_cache["nc"]
    nc = bacc.Bacc("TRN2", target_bir_lowering=False)

    xT_d = nc.declare_dram_parameter("xT", [128, SLOTS], F32R, isOutput=False)
    ea_d = nc.declare_dram_parameter("ea", [128, TT * D], BF16, isOutput=False)
    mk_d = nc.declare_dram_parameter("mk", [128, TT * D], FP8, isOutput=False)
    w1_d = nc.declare_dram_parameter("w1", [2 * D, DH], F32R, isOutput=False)
    w2_d = nc.declare_dram_parameter("w2", [DH, DH], F32R, isOutput=False)
    w3_d = nc.declare_dram_parameter("w3", [DH, D], F32R, isOutput=False)
    b1_d = nc.declare_dram_parameter("b1", [128, 4], F32, isOutput=False)
    b2_d = nc.declare_dram_parameter("b2", [128, 4], F32, isOutput=False)
    b3_d = nc.declare_dram_parameter("b3", [128, 1], F32, isOutput=False)
    out_d = nc.declare_dram_parameter("outT", [128, SLOTS], F32, isOutput=True)

    RELU = mybir.ActivationFunctionType.Relu
    ADD = mybir.AluOpType.add

    n_chunks = len(CHUNKS)
    BW = T_BLK * D  # columns per block slab

    with tile.TileContext(nc) as tc:
        with (
            tc.tile_pool(name="pers", bufs=1) as pers,
            tc.tile_pool(name="eap", bufs=10) as eap,
            tc.tile_pool(name="mkp", bufs=10) as mkp,
            tc.tile_pool(name="actp", bufs=2) as actp,
            tc.tile_pool(name="scat_ps", bufs=4, space="PSUM") as scat_ps,
            tc.tile_pool(name="mlp_ps", bufs=4, space="PSUM") as mlp_ps,
        ):
            # --- edge/mask slabs for the first blocks go first on the SP ring ---
            ea_ts = {}
            mk_ts = {}

            def load_block(b):
                ea_t = eap.tile([128, BW], BF16, tag="ea")
                nc.sync.dma_start(out=ea_t[:], in_=ea_d[:, b * BW : (b + 1) * BW])
                mk_t = mkp.tile([128, BW], FP8, tag="mk")
                nc.sync.dma_start(out=mk_t[:], in_=mk_d[:, b * BW : (b + 1) * BW])
                ea_ts[b] = ea_t
                mk_ts[b] = mk_t

            for b in range(4):
                load_block(b)

            # --- persistent tiles on the ACT HWDGE ring ---
            b1t = pers.tile([128, 4], F32)
            nc.scalar.dma_start(out=b1t[:], in_=b1_d[:])
            b2t = pers.tile([128, 4], F32)
            nc.scalar.dma_start(out=b2t[:], in_=b2_d[:])
            b3t = pers.tile([128, 1], F32)
            nc.scalar.dma_start(out=b3t[:], in_=b3_d[:])
            w1t = pers.tile([128, 2, DH], F32R)
            nc.scalar.dma_start(out=w1t[:], in_=w1_d[:].rearrange("(k p) m -> p k m", p=128))
            # per-chunk x slices (separate tiles so consumers wait per chunk)
            xT_ts = []
            for c_i, nb in enumerate(CHUNKS):
                xt = pers.tile([128, 512], F32R, tag=f"xT{c_i}")
                xT_ts.append(xt)
            nc.scalar.dma_start(out=xT_ts[0][:, : CHUNKS[0] * 128], in_=xT_d[:, 0 : CHUNKS[0] * 128])
            for b in range(4, 6):
                load_block(b)
            w2t = pers.tile([128, 4, DH], F32R)
            nc.scalar.dma_start(out=w2t[:], in_=w2_d[:].rearrange("(k p) m -> p k m", p=128))
            for b in range(6, 8):
                load_block(b)
            w3t = pers.tile([128, 4, D], F32R)
            nc.scalar.dma_start(out=w3t[:], in_=w3_d[:].rearrange("(k p) m -> p k m", p=128))
            xT_cols = [0]
            for c_i in range(1, n_chunks):
                xT_cols.append(xT_cols[-1] + CHUNKS[c_i - 1] * 128)
            nc.scalar.dma_start(
                out=xT_ts[1][:, : CHUNKS[1] * 128],
                in_=xT_d[:, xT_cols[1] : xT_cols[1] + CHUNKS[1] * 128],
            )

            blk0 = 0
            col0 = 0
            next_load = 8
            for c_i, nb in enumerate(CHUNKS):
                NCW = nb * 128
                # scatter per block: segment-mean via mask matmuls
                mean_t = actp.tile([128, 512], F32R, tag="mean")
                for b in range(nb):
                    blk = blk0 + b
                    ps_s = scat_ps.tile([128, 128], F32, tag="scat")
                    ea_t, mk_t = ea_ts.pop(blk), mk_ts.pop(blk)
                    for t in range(T_BLK):
                        j = t * D
                        nc.tensor.matmul(
                            out=ps_s[:],
                            lhsT=ea_t[:, j : j + D],
                            rhs=mk_t[:, j : j + D],
                            start=(t == 0),
                            stop=(t == T_BLK - 1),
                        )
                    # prefetch a later block's slabs
                    if next_load < BPC:
                        load_block(next_load)
                        next_load += 1
                    nc.vector.tensor_copy(out=mean_t[:, b * 128 : (b + 1) * 128], in_=ps_s[:])

                # layer 1: h1 = relu(W1.T @ [x; mean] + b1)
                h1_t = actp.tile([128, 4, 512], F32R, tag="h1")
                for m in range(4):
                    pm = mlp_ps.tile([128, 512], F32, tag="mlp")
                    nc.tensor.matmul(
                        out=pm[:, :NCW],
                        lhsT=w1t[:, 1, m * 128 : (m + 1) * 128],
                        rhs=mean_t[:, :NCW],
                        start=True,
                        stop=False,
                    )
                    nc.tensor.matmul(
                        out=pm[:, :NCW],
                        lhsT=w1t[:, 0, m * 128 : (m + 1) * 128],
                        rhs=xT_ts[c_i][:, :NCW],
                        start=False,
                        stop=True,
                    )
                    nc.scalar.activation(
                        out=h1_t[:, m, :NCW], in_=pm[:, :NCW], func=RELU, bias=b1t[:, m : m + 1]
                    )
                # layer 2
                h2_t = actp.tile([128, 4, 512], F32R, tag="h2")
                for m in range(4):
                    pm = mlp_ps.tile([128, 512], F32, tag="mlp")
                    for k in range(4):
                        nc.tensor.matmul(
                            out=pm[:, :NCW],
                            lhsT=w2t[:, k, m * 128 : (m + 1) * 128],
                            rhs=h1_t[:, k, :NCW],
                            start=(k == 0),
                            stop=(k == 3),
                        )
                    nc.scalar.activation(
                        out=h2_t[:, m, :NCW], in_=pm[:, :NCW], func=RELU, bias=b2t[:, m : m + 1]
                    )
                # layer 3: out = W3.T @ h2 + b3  (bias add on DVE)
                pm = mlp_ps.tile([128, 512], F32, tag="mlp")
                for k in range(4):
                    nc.tensor.matmul(
                        out=pm[:, :NCW],
                        lhsT=w3t[:, k, :],
                        rhs=h2_t[:, k, :NCW],
                        start=(k == 0),
                        stop=(k == 3),
                    )
                out_t = actp.tile([128, 512], F32, tag="out")
                nc.vector.tensor_scalar_add(out_t[:, :NCW], pm[:, :NCW], b3t[:, 0:1])
                nc.scalar.dma_start(out=out_d[:, col0 : col0 + NCW], in_=out_t[:, :NCW])

                if c_i + 2 < n_chunks:
                    ncw2 = CHUNKS[c_i + 2] * 128
                    nc.scalar.dma_start(
                        out=xT_ts[c_i + 2][:, :ncw2],
                        in_=xT_d[:, xT_cols[c_i + 2] : xT_cols[c_i + 2] + ncw2],
                    )
                blk0 += nb
                col0 += NCW

    nc.compile()
    _prog_cache["nc"] = nc
    return nc


def _preprocess(x, edge_index, edge_attr):
    recv = np.asarray(edge_index)[1].astype(np.int64)
    deg = np.bincount(recv, minlength=N_NODES)
    # snake assignment of degree-sorted nodes into NB blocks (125 nodes/block)
    order = np.argsort(-deg, kind="stable")
    i = np.arange(N_NODES)
    rnd, pos = i // NB, i % NB
    blk = np.where(rnd % 2 == 0, pos, NB - 1 - pos)
    node_block = np.empty(N_NODES, np.int64)
    node_slot = np.empty(N_NODES, np.int64)
    node_block[order] = blk
    node_slot[order] = rnd
    node_core = node_block // BPC
    node_col = (node_block % BPC) * 128 + node_slot

    eb = node_block[recv]
    bc = np.bincount(eb, minlength=NB)
    if bc.max() > T_BLK * 128:
        raise RuntimeError(f"block overflow: {bc.max()} > {T_BLK * 128}")

    eorder = np.argsort(eb, kind="stable")
    eb_s = eb[eorder]
    starts = np.zeros(NB, np.int64)
    starts[1:] = np.cumsum(bc)[:-1]
    ewithin = np.arange(N_EDGES) - starts[eb_s]
    ktile = ewithin // 128
    eslot = ewithin % 128
    ecore = eb_s // BPC
    kt_in_core = (eb_s % BPC) * T_BLK + ktile

    # scale edges by 1/deg(recv) on the host, then cast once to bf16
    ea_scaled = np.asarray(edge_attr, np.float32) * (1.0 / deg[recv])[:, None].astype(np.float32)
    ea_bf = ea_scaled.astype(ml_dtypes.bfloat16)
    ea_buf = np.zeros((C, TT, 128, D), ml_dtypes.bfloat16)
    ea_buf[ecore, kt_in_core, eslot] = ea_bf[eorder]
    # 0/1 scatter masks in fp8
    mk_buf = np.zeros((C, TT, 128, D), ml_dtypes.float8_e4m3)
    mk_buf[ecore, kt_in_core, eslot, (node_col[recv] % 128)[eorder]] = 1.0

    X_all = np.zeros((C, SLOTS, D), np.float32)
    X_all[node_core, node_col] = np.asarray(x, np.float32)

    shards = []
    for c in range(C):
        shards.append(
            dict(
                xT=np.ascontiguousarray(X_all[c].T),
                ea=np.ascontiguousarray(ea_buf[c].transpose(1, 0, 2).reshape(128, TT * D)),
                mk=np.ascontiguousarray(mk_buf[c].transpose(1, 0, 2).reshape(128, TT * D)),
            )
        )
    return shards, node_core, node_col


def kernel(x, edge_index, edge_attr, W1, b1, W2, b2, W3, b3, _trace=False):
    global LAST_RESULTS
    shards, node_core, node_col = _preprocess(x, edge_index, edge_attr)

    W1 = np.ascontiguousarray(np.asarray(W1, np.float32))
    W2 = np.ascontiguousarray(np.asarray(W2, np.float32))
    W3 = np.ascontiguousarray(np.asarray(W3, np.float32))
    b1r = np.ascontiguousarray(np.asarray(b1, np.float32).reshape(4, 128).T)
    b2r = np.ascontiguousarray(np.asarray(b2, np.float32).reshape(4, 128).T)
    b3r = np.ascontiguousarray(np.asarray(b3, np.float32).reshape(1, 128).T)

    in_maps = []
    for c in range(C):
        m = dict(shards[c])
        m.update(w1=W1, w2=W2, w3=W3, b1=b1r, b2=b2r, b3=b3r)
        in_maps.append(m)

    nc = _build_program()
    res = run_bass_kernel_spmd(nc, in_maps, core_ids=list(range(C)), trace=_trace)
    LAST_RESULTS = res

    outs = np.stack([res.results[c]["outT"] for c in range(C)])  # [C, 128, SLOTS]
    out = outs.transpose(0, 2, 1)[node_core, node_col]
    return np.ascontiguousarray(out, dtype=np.float32)


# revision 9
# speedup vs baseline: 1.0256x; 1.0256x over previous
"""GNN message-passing (segment-mean + 3-layer MLP) Trainium2 kernel.

Strategy (8 NeuronCores, SPMD, full inputs in / full output out):
  - Host: assign nodes to 400 blocks of 128 slots (degree-balanced snake) so
    every block's incoming-edge count fits a fixed 12 k-tiles of 128 edges.
    Blocks 0-49 -> core 0, etc.  Edges are bucketed per receiver block,
    pre-scaled by 1/deg(recv), cast to bf16, and laid out [eslot, ktile*feat]
    so per-block DMAs are contiguous.  The one-hot scatter masks (0/1, fp8)
    are precomputed on the host as well.
  - Device per core: segment-mean as mask matmuls on the TensorEngine
    (12 k-tiles accumulated per 128-node block), then the 3-layer MLP over
    512-node chunks in feature-major layout with float32r matmuls
    (full-rate fp32-storage), relu+bias fused on the ScalarEngine, psum
    copies / final bias on the VectorEngine.  Edge/mask slabs stream on the
    SP HWDGE ring while weights/x stream on the ACT ring so the first
    scatter matmul is not queued behind the persistent loads.
    Output stays transposed; host untransposes + un-permutes.
"""
import sys

sys.path.insert(0, "/opt/trn_rl_repo")

import numpy as np
import ml_dtypes

from concourse import bacc
import concourse.mybir as mybir
import concourse.tile as tile
from concourse.bass_utils import run_bass_kernel_spmd

# problem shape (hardcoded per contract)
N_NODES = 50000
N_EDGES = 600000
D = 128          # node/edge feature dim
DH = 512         # hidden dim
C = 8            # cores
BPC = 50         # node blocks per core
NB = C * BPC     # 400 blocks total
SLOTS = BPC * 128  # 6400 node slots per core
T_BLK = 12       # edge k-tiles (128 edges) per block
TT = BPC * T_BLK   # k-tiles per core
CHUNKS = [4] * 12 + [2]  # blocks per MLP chunk (512/256 nodes)

F32 = mybir.dt.float32
F32R = mybir.dt.float32r
BF16 = mybir.dt.bfloat16
FP8 = mybir.dt.float8e4

_prog_cache = {}
LAST_RESULTS = None  # BassKernelResults of the most recent run (for test.py)


def _build_program():
    if "nc" in _prog_cache:
        return _prog_cache["nc"]
    nc = bacc.Bacc("TRN2", target_bir_lowering=False)

    xT_d = nc.declare_dram_parameter("xT", [128, SLOTS], F32R, isOutput=False)
    ea_d = nc.declare_dram_parameter("ea", [128, TT * D], BF16, isOutput=False)
    mk_d = nc.declare_dram_parameter("mk", [128, TT * D], FP8, isOutput=False)
    w1_d = nc.declare_dram_parameter("w1", [2 * D, DH], F32R, isOutput=False)
    w2_d = nc.declare_dram_parameter("w2", [DH, DH], F32R, isOutput=False)
    w3_d = nc.declare_dram_parameter("w3", [DH, D], F32R, isOutput=False)
    b1_d = nc.declare_dram_parameter("b1", [128, 4], F32, isOutput=False)
    b2_d = nc.declare_dram_parameter("b2", [128, 4], F32, isOutput=False)
    b3_d = nc.declare_dram_parameter("b3", [128, 1], F32, isOutput=False)
    out_d = nc.declare_dram_parameter("outT", [128, SLOTS], F32, isOutput=True)

    RELU = mybir.ActivationFunctionType.Relu
    ADD = mybir.AluOpType.add

    n_chunks = len(CHUNKS)
    BW = T_BLK * D  # columns per block slab

    with tile.TileContext(nc) as tc:
        with (
            tc.tile_pool(name="pers", bufs=1) as pers,
            tc.tile_pool(name="eap", bufs=10) as eap,
            tc.tile_pool(name="mkp", bufs=10) as mkp,
            tc.tile_pool(name="actp", bufs=2) as actp,
            tc.tile_pool(name="scat_ps", bufs=4, space="PSUM") as scat_ps,
            tc.tile_pool(name="mlp_ps", bufs=4, space="PSUM") as mlp_ps,
        ):
            # --- edge/mask slabs for the first blocks go first on the SP ring ---
            ea_ts = {}
            mk_ts = {}

            def load_block(b):
                ea_t = eap.tile([128, BW], BF16, tag="ea")
                nc.sync.dma_start(out=ea_t[:], in_=ea_d[:, b * BW : (b + 1) * BW])
                mk_t = mkp.tile([128, BW], FP8, tag="mk")
                nc.sync.dma_start(out=mk_t[:], in_=mk_d[:, b * BW : (b + 1) * BW])
                ea_ts[b] = ea_t
                mk_ts[b] = mk_t

            for b in range(4):
                load_block(b)

            # --- persistent tiles on the ACT HWDGE ring ---
            b1t = pers.tile([128, 4], F32)
            nc.scalar.dma_start(out=b1t[:], in_=b1_d[:])
            b2t = pers.tile([128, 4], F32)
            nc.scalar.dma_start(out=b2t[:], in_=b2_d[:])
            b3t = pers.tile([128, 1], F32)
            nc.scalar.dma_start(out=b3t[:], in_=b3_d[:])
            w1t = pers.tile([128, 2, DH], F32R)
            nc.scalar.dma_start(out=w1t[:], in_=w1_d[:].rearrange("(k p) m -> p k m", p=128))
            # per-chunk x slices (separate tiles so consumers wait per chunk)
            xT_ts = []
            for c_i, nb in enumerate(CHUNKS):
                xt = pers.tile([128, 512], F32R, tag=f"xT{c_i}")
                xT_ts.append(xt)
            nc.scalar.dma_start(out=xT_ts[0][:, : CHUNKS[0] * 128], in_=xT_d[:, 0 : CHUNKS[0] * 128])
            for b in range(4, 6):
                load_block(b)
            w2t = pers.tile([128, 4, DH], F32R)
            nc.scalar.dma_start(out=w2t[:], in_=w2_d[:].rearrange("(k p) m -> p k m", p=128))
            for b in range(6, 8):
                load_block(b)
            w3t = pers.tile([128, 4, D], F32R)
            nc.scalar.dma_start(out=w3t[:], in_=w3_d[:].rearrange("(k p) m -> p k m", p=128))
            xT_cols = [0]
            for c_i in range(1, n_chunks):
                xT_cols.append(xT_cols[-1] + CHUNKS[c_i - 1] * 128)
            nc.scalar.dma_start(
                out=xT_ts[1][:, : CHUNKS[1] * 128],
                in_=xT_d[:, xT_cols[1] : xT_cols[1] + CHUNKS[1] * 128],
            )

            blk0 = 0
            col0 = 0
            next_load = 8
            for c_i, nb in enumerate(CHUNKS):
                NCW = nb * 128
                # scatter per block: segment-mean via mask matmuls
                mean_t = actp.tile([128, 512], F32R, tag="mean")
                for b in range(nb):
                    blk = blk0 + b
                    ps_s = scat_ps.tile([128, 128], F32, tag="scat")
                    ea_t, mk_t = ea_ts.pop(blk), mk_ts.pop(blk)
                    for t in range(T_BLK):
                        j = t * D
                        nc.tensor.matmul(
                            out=ps_s[:],
                            lhsT=ea_t[:, j : j + D],
                            rhs=mk_t[:, j : j + D],
                            start=(t == 0),
                            stop=(t == T_BLK - 1),
                        )
                    # prefetch a later block's slabs
                    if next_load < BPC:
                        load_block(next_load)
                        next_load += 1
                    nc.vector.tensor_copy(out=mean_t[:, b * 128 : (b + 1) * 128], in_=ps_s[:])

                # layer 1: h1 = relu(W1.T @ [x; mean] + b1)
                h1_t = actp.tile([128, 4, 512], F32R, tag="h1")
                for m in range(4):
                    pm = mlp_ps.tile([128, 512], F32, tag="mlp")
                    nc.tensor.matmul(
                        out=pm[:, :NCW],
                        lhsT=w1t[:, 1, m * 128 : (m + 1) * 128],
                        rhs=mean_t[:, :NCW],
                        start=True,
                        stop=False,
                    )
                    nc.tensor.matmul(
                        out=pm[:, :NCW],
                        lhsT=w1t[:, 0, m * 128 : (m + 1) * 128],
                        rhs=xT_ts[c_i][:, :NCW],
                        start=False,
                        stop=True,
                    )
                    nc.scalar.activation(
                        out=h1_t[:, m, :NCW], in_=pm[:, :NCW], func=RELU, bias=b1t[:, m : m + 1]
                    )
                # layer 2
                h2_t = actp.tile([128, 4, 512], F32R, tag="h2")
                for m in range(4):
                    pm = mlp_ps.tile([128, 512], F32, tag="mlp")
                    for k in range(4):
                        nc.tensor.matmul(
                            out=pm[:, :NCW],
                            lhsT=w2t[:, k, m * 128 : (m + 1) * 128],
                            rhs=h1_t[:, k, :NCW],
                            start=(k == 0),
                            stop=(k == 3),
                        )
                    nc.scalar.activation(
                        out=h2_t[:, m, :NCW], in_=pm[:, :NCW], func=RELU, bias=b2t[:, m : m + 1]
                    )
                # layer 3: out = W3.T @ h2 + b3  (bias add on DVE)
                pm = mlp_ps.tile([128, 512], F32, tag="mlp")
                for k in range(4):
                    nc.tensor.matmul(
                        out=pm[:, :NCW],
                        lhsT=w3t[:, k, :],
                        rhs=h2_t[:, k, :NCW],
                        start=(k == 0),
                        stop=(k == 3),
                    )
                out_t = actp.tile([128, 512], F32, tag="out")
                nc.vector.tensor_scalar_add(out_t[:, :NCW], pm[:, :NCW], b3t[:, 0:1])
                nc.scalar.dma_start(out=out_d[:, col0 : col0 + NCW], in_=out_t[:, :NCW])

                if c_i + 2 < n_chunks:
                    ncw2 = CHUNKS[c_i + 2] * 128
                    nc.scalar.dma_start(
                        out=xT_ts[c_i + 2][:, :ncw2],
                        in_=xT_d[:, xT_cols[c_i + 2] : xT_cols[c_i + 2] + ncw2],
                    )
                blk0 += nb
                col0 += NCW

    nc.compile()
    _prog_cache["nc"] = nc
    return nc


def _preprocess(x, edge_index, edge_attr):
    recv = np.asarray(edge_index)[1].astype(np.int64)
    deg = np.bincount(recv, minlength=N_NODES)
    # snake assignment of degree-sorted nodes into NB blocks (125 nodes/block)
    order = np.argsort(-deg, kind="stable")
    i = np.arange(N_NODES)
    rnd, pos = i // NB, i % NB
    blk = np.where(rnd % 2 == 0, pos, NB - 1 - pos)
    node_block = np.empty(N_NODES, np.int64)
    node_slot = np.empty(N_NODES, np.int64)
    node_block[order] = blk
    node_slot[order] = rnd
    node_core = node_block // BPC
    node_col = (node_block % BPC) * 128 + node_slot

    eb = node_block[recv]
    bc = np.bincount(eb, minlength=NB)
    if bc.max() > T_BLK * 128:
        raise RuntimeError(f"block overflow: {bc.max()} > {T_BLK * 128}")

    eorder = np.argsort(eb, kind="stable")
    eb_s = eb[eorder]
    starts = np.zeros(NB, np.int64)
    starts[1:] = np.cumsum(bc)[:-1]
    ewithin = np.arange(N_EDGES) - starts[eb_s]
    ktile = ewithin // 128
    eslot = ewithin % 128
    ecore = eb_s // BPC
    kt_in_core = (eb_s % BPC) * T_BLK + ktile

    # scale edges by 1/deg(recv) on the host, then cast once to bf16
    ea_scaled = np.asarray(edge_attr, np.float32) * (1.0 / deg[recv])[:, None].astype(np.float32)
    ea_bf = ea_scaled.astype(ml_dtypes.bfloat16)
    ea_buf = np.zeros((C, TT, 128, D), ml_dtypes.bfloat16)
    ea_buf[ecore, kt_in_core, eslot] = ea_bf[eorder]
    # 0/1 scatter masks in fp8
    mk_buf = np.zeros((C, TT, 128, D), ml_dtypes.float8_e4m3)
    mk_buf[ecore, kt_in_core, eslot, (node_col[recv] % 128)[eorder]] = 1.0

    X_all = np.zeros((C, SLOTS, D), np.float32)
    X_all[node_core, node_col] = np.asarray(x, np.float32)

    shards = []
    for c in range(C):
        shards.append(
            dict(
                xT=np.ascontiguousarray(X_all[c].T),
                ea=np.ascontiguousarray(ea_buf[c].transpose(1, 0, 2).reshape(128, TT * D)),
                mk=np.ascontiguousarray(mk_buf[c].transpose(1, 0, 2).reshape(128, TT * D)),
            )
        )
    return shards, node_core, node_col


def kernel(x, edge_index, edge_attr, W1, b1, W2, b2, W3, b3, _trace=False):
    global LAST_RESULTS
    shards, node_core, node_col = _preprocess(x, edge_index, edge_attr)

    W1 = np.ascontiguousarray(np.asarray(W1, np.float32))
    W2 = np.ascontiguousarray(np.asarray(W2, np.float32))
    W3 = np.ascontiguousarray(np.asarray(W3, np.float32))
    b1r = np.ascontiguousarray(np.asarray(b1, np.float32).reshape(4, 128).T)
    b2r = np.ascontiguousarray(np.asarray(b2, np.float32).reshape(4, 128).T)
    b3r = np.ascontiguousarray(np.asarray(b3, np.float32).reshape(1, 128).T)

    in_maps = []
    for c in range(C):
        m = dict(shards[c])
        m.update(w1=W1, w2=W2, w3=W3, b1=b1r, b2=b2r, b3=b3r)
        in_maps.append(m)

    nc = _build_program()
    res = run_bass_kernel_spmd(nc, in_maps, core_ids=list(range(C)), trace=_trace)
    LAST_RESULTS = res

    outs = np.stack([res.results[c]["outT"] for c in range(C)])  # [C, 128, SLOTS]
    out = outs.transpose(0, 2, 1)[node_core, node_col]
    return np.ascontiguousarray(out, dtype=np.float32)


# revision 10
# speedup vs baseline: 1.0437x; 1.0177x over previous
"""GNN message-passing (segment-mean + 3-layer MLP) Trainium2 kernel.

Strategy (8 NeuronCores, SPMD, full inputs in / full output out):
  - Host: assign nodes to 400 blocks of 128 slots (degree-balanced snake) so
    every block's incoming-edge count fits a fixed 12 k-tiles of 128 edges.
    Blocks 0-49 -> core 0, etc.  Edges are bucketed per receiver block,
    pre-scaled by 1/deg(recv), cast to bf16, and laid out [eslot, ktile*feat]
    so per-block DMAs are contiguous.  The one-hot scatter masks (0/1, fp8)
    are precomputed on the host as well.
  - Device per core: segment-mean as mask matmuls on the TensorEngine
    (12 k-tiles accumulated per 128-node block), then the 3-layer MLP over
    512-node chunks in feature-major layout with float32r matmuls
    (full-rate fp32-storage), relu+bias fused on the ScalarEngine, psum
    copies / final bias on the VectorEngine.  Edge/mask slabs stream on the
    SP HWDGE ring while weights/x stream on the ACT ring so the first
    scatter matmul is not queued behind the persistent loads.
    Output stays transposed; host untransposes + un-permutes.
"""
import sys

sys.path.insert(0, "/opt/trn_rl_repo")

import numpy as np
import ml_dtypes

from concourse import bacc
import concourse.mybir as mybir
import concourse.tile as tile
from concourse.bass_utils import run_bass_kernel_spmd

# problem shape (hardcoded per contract)
N_NODES = 50000
N_EDGES = 600000
D = 128          # node/edge feature dim
DH = 512         # hidden dim
C = 8            # cores
BPC = 50         # node blocks per core
NB = C * BPC     # 400 blocks total
SLOTS = BPC * 128  # 6400 node slots per core
T_BLK = 12       # edge k-tiles (128 edges) per block
TT = BPC * T_BLK   # k-tiles per core
CHUNKS = [4] * 12 + [2]  # blocks per MLP chunk (512/256 nodes)

F32 = mybir.dt.float32
F32R = mybir.dt.float32r
BF16 = mybir.dt.bfloat16
FP8 = mybir.dt.float8e4

_prog_cache = {}
LAST_RESULTS = None  # BassKernelResults of the most recent run (for test.py)


def _build_program():
    if "nc" in _prog_cache:
        return _prog_cache["nc"]
    nc = bacc.Bacc("TRN2", target_bir_lowering=False)

    xT_d = nc.declare_dram_parameter("xT", [128, SLOTS], F32R, isOutput=False)
    ea_d = nc.declare_dram_parameter("ea", [128, TT * D], BF16, isOutput=False)
    mk_d = nc.declare_dram_parameter("mk", [128, TT * D], FP8, isOutput=False)
    w1_d = nc.declare_dram_parameter("w1", [2 * D, DH], F32R, isOutput=False)
    w2_d = nc.declare_dram_parameter("w2", [DH, DH], F32R, isOutput=False)
    w3_d = nc.declare_dram_parameter("w3", [DH, D], F32R, isOutput=False)
    b1_d = nc.declare_dram_parameter("b1", [128, 4], F32, isOutput=False)
    b2_d = nc.declare_dram_parameter("b2", [128, 4], F32, isOutput=False)
    b3_d = nc.declare_dram_parameter("b3", [128, 1], F32, isOutput=False)
    out_d = nc.declare_dram_parameter("outT", [128, SLOTS], F32, isOutput=True)

    RELU = mybir.ActivationFunctionType.Relu
    ADD = mybir.AluOpType.add

    n_chunks = len(CHUNKS)
    BW = T_BLK * D  # columns per block slab

    with tile.TileContext(nc) as tc:
        with (
            tc.tile_pool(name="pers", bufs=1) as pers,
            tc.tile_pool(name="eap", bufs=12) as eap,
            tc.tile_pool(name="mkp", bufs=12) as mkp,
            tc.tile_pool(name="actp", bufs=3) as actp,
            tc.tile_pool(name="scat_ps", bufs=4, space="PSUM") as scat_ps,
            tc.tile_pool(name="mlp_ps", bufs=4, space="PSUM") as mlp_ps,
        ):
            # --- edge/mask slabs for the first blocks go first on the SP ring ---
            ea_ts = {}
            mk_ts = {}

            def load_block(b):
                ea_t = eap.tile([128, BW], BF16, tag="ea")
                nc.sync.dma_start(out=ea_t[:], in_=ea_d[:, b * BW : (b + 1) * BW])
                mk_t = mkp.tile([128, BW], FP8, tag="mk")
                nc.sync.dma_start(out=mk_t[:], in_=mk_d[:, b * BW : (b + 1) * BW])
                ea_ts[b] = ea_t
                mk_ts[b] = mk_t

            for b in range(6):
                load_block(b)

            # --- persistent tiles on the ACT HWDGE ring ---
            b1t = pers.tile([128, 4], F32)
            nc.scalar.dma_start(out=b1t[:], in_=b1_d[:])
            b2t = pers.tile([128, 4], F32)
            nc.scalar.dma_start(out=b2t[:], in_=b2_d[:])
            b3t = pers.tile([128, 1], F32)
            nc.scalar.dma_start(out=b3t[:], in_=b3_d[:])
            w1t = pers.tile([128, 2, DH], F32R)
            nc.scalar.dma_start(out=w1t[:], in_=w1_d[:].rearrange("(k p) m -> p k m", p=128))
            # per-chunk x slices (separate tiles so consumers wait per chunk)
            xT_ts = []
            for c_i, nb in enumerate(CHUNKS):
                xt = pers.tile([128, 512], F32R, tag=f"xT{c_i}")
                xT_ts.append(xt)
            nc.scalar.dma_start(out=xT_ts[0][:, : CHUNKS[0] * 128], in_=xT_d[:, 0 : CHUNKS[0] * 128])
            for b in range(6, 8):
                load_block(b)
            w2t = pers.tile([128, 4, DH], F32R)
            nc.scalar.dma_start(out=w2t[:], in_=w2_d[:].rearrange("(k p) m -> p k m", p=128))
            for b in range(8, 10):
                load_block(b)
            w3t = pers.tile([128, 4, D], F32R)
            nc.scalar.dma_start(out=w3t[:], in_=w3_d[:].rearrange("(k p) m -> p k m", p=128))
            xT_cols = [0]
            for c_i in range(1, n_chunks):
                xT_cols.append(xT_cols[-1] + CHUNKS[c_i - 1] * 128)
            nc.scalar.dma_start(
                out=xT_ts[1][:, : CHUNKS[1] * 128],
                in_=xT_d[:, xT_cols[1] : xT_cols[1] + CHUNKS[1] * 128],
            )

            blk0 = 0
            col0 = 0
            next_load = 10
            for c_i, nb in enumerate(CHUNKS):
                NCW = nb * 128
                # scatter per block: segment-mean via mask matmuls
                mean_t = actp.tile([128, 512], F32R, tag="mean")
                for b in range(nb):
                    blk = blk0 + b
                    ps_s = scat_ps.tile([128, 128], F32, tag="scat")
                    ea_t, mk_t = ea_ts.pop(blk), mk_ts.pop(blk)
                    for t in range(T_BLK):
                        j = t * D
                        nc.tensor.matmul(
                            out=ps_s[:],
                            lhsT=ea_t[:, j : j + D],
                            rhs=mk_t[:, j : j + D],
                            start=(t == 0),
                            stop=(t == T_BLK - 1),
                        )
                    # prefetch a later block's slabs
                    if next_load < BPC:
                        load_block(next_load)
                        next_load += 1
                    nc.vector.tensor_copy(out=mean_t[:, b * 128 : (b + 1) * 128], in_=ps_s[:])

                # layer 1: h1 = relu(W1.T @ [x; mean] + b1)
                h1_t = actp.tile([128, 4, 512], F32R, tag="h1")
                for m in range(4):
                    pm = mlp_ps.tile([128, 512], F32, tag="mlp")
                    nc.tensor.matmul(
                        out=pm[:, :NCW],
                        lhsT=w1t[:, 1, m * 128 : (m + 1) * 128],
                        rhs=mean_t[:, :NCW],
                        start=True,
                        stop=False,
                    )
                    nc.tensor.matmul(
                        out=pm[:, :NCW],
                        lhsT=w1t[:, 0, m * 128 : (m + 1) * 128],
                        rhs=xT_ts[c_i][:, :NCW],
                        start=False,
                        stop=True,
                    )
                    nc.scalar.activation(
                        out=h1_t[:, m, :NCW], in_=pm[:, :NCW], func=RELU, bias=b1t[:, m : m + 1]
                    )
                # layer 2
                h2_t = actp.tile([128, 4, 512], F32R, tag="h2")
                for m in range(4):
                    pm = mlp_ps.tile([128, 512], F32, tag="mlp")
                    for k in range(4):
                        nc.tensor.matmul(
                            out=pm[:, :NCW],
                            lhsT=w2t[:, k, m * 128 : (m + 1) * 128],
                            rhs=h1_t[:, k, :NCW],
                            start=(k == 0),
                            stop=(k == 3),
                        )
                    nc.scalar.activation(
                        out=h2_t[:, m, :NCW], in_=pm[:, :NCW], func=RELU, bias=b2t[:, m : m + 1]
                    )
                # layer 3: out = W3.T @ h2 + b3  (bias add on DVE)
                pm = mlp_ps.tile([128, 512], F32, tag="mlp")
                for k in range(4):
                    nc.tensor.matmul(
                        out=pm[:, :NCW],
                        lhsT=w3t[:, k, :],
                        rhs=h2_t[:, k, :NCW],
                        start=(k == 0),
                        stop=(k == 3),
                    )
                out_t = actp.tile([128, 512], F32, tag="out")
                nc.vector.tensor_scalar_add(out_t[:, :NCW], pm[:, :NCW], b3t[:, 0:1])
                nc.scalar.dma_start(out=out_d[:, col0 : col0 + NCW], in_=out_t[:, :NCW])

                if c_i + 2 < n_chunks:
                    ncw2 = CHUNKS[c_i + 2] * 128
                    nc.scalar.dma_start(
                        out=xT_ts[c_i + 2][:, :ncw2],
                        in_=xT_d[:, xT_cols[c_i + 2] : xT_cols[c_i + 2] + ncw2],
                    )
                blk0 += nb
                col0 += NCW

    nc.compile()
    _prog_cache["nc"] = nc
    return nc


def _preprocess(x, edge_index, edge_attr):
    recv = np.asarray(edge_index)[1].astype(np.int64)
    deg = np.bincount(recv, minlength=N_NODES)
    # snake assignment of degree-sorted nodes into NB blocks (125 nodes/block)
    order = np.argsort(-deg, kind="stable")
    i = np.arange(N_NODES)
    rnd, pos = i // NB, i % NB
    blk = np.where(rnd % 2 == 0, pos, NB - 1 - pos)
    node_block = np.empty(N_NODES, np.int64)
    node_slot = np.empty(N_NODES, np.int64)
    node_block[order] = blk
    node_slot[order] = rnd
    node_core = node_block // BPC
    node_col = (node_block % BPC) * 128 + node_slot

    eb = node_block[recv]
    bc = np.bincount(eb, minlength=NB)
    if bc.max() > T_BLK * 128:
        raise RuntimeError(f"block overflow: {bc.max()} > {T_BLK * 128}")

    eorder = np.argsort(eb, kind="stable")
    eb_s = eb[eorder]
    starts = np.zeros(NB, np.int64)
    starts[1:] = np.cumsum(bc)[:-1]
    ewithin = np.arange(N_EDGES) - starts[eb_s]
    ktile = ewithin // 128
    eslot = ewithin % 128
    ecore = eb_s // BPC
    kt_in_core = (eb_s % BPC) * T_BLK + ktile

    # scale edges by 1/deg(recv) on the host, then cast once to bf16
    ea_scaled = np.asarray(edge_attr, np.float32) * (1.0 / deg[recv])[:, None].astype(np.float32)
    ea_bf = ea_scaled.astype(ml_dtypes.bfloat16)
    ea_buf = np.zeros((C, TT, 128, D), ml_dtypes.bfloat16)
    ea_buf[ecore, kt_in_core, eslot] = ea_bf[eorder]
    # 0/1 scatter masks in fp8
    mk_buf = np.zeros((C, TT, 128, D), ml_dtypes.float8_e4m3)
    mk_buf[ecore, kt_in_core, eslot, (node_col[recv] % 128)[eorder]] = 1.0

    X_all = np.zeros((C, SLOTS, D), np.float32)
    X_all[node_core, node_col] = np.asarray(x, np.float32)

    shards = []
    for c in range(C):
        shards.append(
            dict(
                xT=np.ascontiguousarray(X_all[c].T),
                ea=np.ascontiguousarray(ea_buf[c].transpose(1, 0, 2).reshape(128, TT * D)),
                mk=np.ascontiguousarray(mk_buf[c].transpose(1, 0, 2).reshape(128, TT * D)),
            )
        )
    return shards, node_core, node_col


def kernel(x, edge_index, edge_attr, W1, b1, W2, b2, W3, b3, _trace=False):
    global LAST_RESULTS
    shards, node_core, node_col = _preprocess(x, edge_index, edge_attr)

    W1 = np.ascontiguousarray(np.asarray(W1, np.float32))
    W2 = np.ascontiguousarray(np.asarray(W2, np.float32))
    W3 = np.ascontiguousarray(np.asarray(W3, np.float32))
    b1r = np.ascontiguousarray(np.asarray(b1, np.float32).reshape(4, 128).T)
    b2r = np.ascontiguousarray(np.asarray(b2, np.float32).reshape(4, 128).T)
    b3r = np.ascontiguousarray(np.asarray(b3, np.float32).reshape(1, 128).T)

    in_maps = []
    for c in range(C):
        m = dict(shards[c])
        m.update(w1=W1, w2=W2, w3=W3, b1=b1r, b2=b2r, b3=b3r)
        in_maps.append(m)

    nc = _build_program()
    res = run_bass_kernel_spmd(nc, in_maps, core_ids=list(range(C)), trace=_trace)
    LAST_RESULTS = res

    outs = np.stack([res.results[c]["outT"] for c in range(C)])  # [C, 128, SLOTS]
    out = outs.transpose(0, 2, 1)[node_core, node_col]
    return np.ascontiguousarray(out, dtype=np.float32)
